# revision 1
# baseline (speedup 1.0000x reference)
"""Trainium2 Bass kernel for nn_Attention (B=2, S=2048, D=512, H=8).

Sharding: 8 cores = 2 batches x 4 head-groups (2 heads each).

Algebraic fusions (exact, host-side weight preprocessing in f64):
  W_full = W_multi @ W_sep  (the two projection layers collapse into one),
  G_h    = Wq_h^T @ Wk_h    so  S_h = (x G_h) x^T   (k-projection vanishes),
  Wvp_h  = (Wres_h @ Wv_h)^T so PV directly yields the output partial
           (restore matmul vanishes).
Bias terms: q-side/constant bias cancels inside softmax; the surviving
k-side term (x Wk^T bq)/sqrt(D) is a per-sk vector folded into the exp's
per-partition bias; V'-bias is a broadcast add; b_res is added on host.

Device compute per core (bf16 matmuls, f32 PSUM, feature-on-partition
layouts, zero on-device transposes):
  qtT = G^T xT                      [D, S]   (per head)
  V'  = x Wvp (+bias)               [S, D]   natural layout
  t3  = x g                         [S]      (exp bias column)
  ST  = x qt^T  -> E = exp(ST/sqrt(D) + t3)  [sk, sq] chunks
  den = E^T ones ; out_h = (E^T V') * 1/den  accumulated over heads into
  out [S, D] f32 (natural layout), summed over cores on host + b_res.
"""

import numpy as np

P = 128
B = 2
S = 2048
D = 512          # word dim == head dim
H = 8            # total heads
E3 = 3 * D       # 1536
NHL = 2          # local heads per core
NC = 8           # cores
CH = 512         # sq chunk width
NCH = S // CH    # 4
NT = S // P      # 16 sk tiles
KD = D // P      # 4
INV_SQRT_D = 1.0 / float(np.sqrt(np.float32(D)))

_CACHE = {}


def _build_nc():
    import concourse.mybir as mybir
    import concourse.tile as tile
    from concourse import bacc

    dt = mybir.dt
    BF = dt.bfloat16
    F32 = dt.float32
    Act = mybir.ActivationFunctionType
    Alu = mybir.AluOpType

    nc = bacc.Bacc("TRN2", target_bir_lowering=False, debug=False, num_devices=NC)

    FP8 = dt.float8e4
    DR = mybir.MatmulPerfMode.DoubleRow
    xT_d = nc.declare_dram_parameter("xT", [D, S], BF, isOutput=False)
    xT8_d = nc.declare_dram_parameter("xT8", [D, S], FP8, isOutput=False)
    g_d = nc.declare_dram_parameter("G", [NHL, D, D], BF, isOutput=False)
    wvp_d = nc.declare_dram_parameter("WvpT", [NHL, D, D], BF, isOutput=False)
    gv_d = nc.declare_dram_parameter("gvec", [NHL, D], BF, isOutput=False)
    bfv_d = nc.declare_dram_parameter("bfv", [NHL, D], F32, isOutput=False)
    out_d = nc.declare_dram_parameter("out", [S, D], F32, isOutput=True)

    with tile.TileContext(nc) as tc:
        with (
            tc.tile_pool(name="w", bufs=1) as wp,
            tc.tile_pool(name="psum", bufs=1, space="PSUM") as pp,
        ):
            ones_sb = wp.tile([P, 1], BF, tag="ones")
            nc.vector.memset(ones_sb[:], 1.0)

            # xT lives as one [P, KD*S] tile (column block kd = d_in k-tile);
            # G / Wvp are one [P, KD*D] tile per head. Each DRAM load is then
            # a single multi-dim-AP DMA, and the first-needed ones issue
            # first (on different engines) so the PE can start early.
            xt = wp.tile([P, KD * S], BF, tag="xt", name="xt")
            gts, wvs, gcol = [], [], []
            for h in range(NHL):
                gts.append(wp.tile([P, KD * D], BF, tag=f"G{h}", name=f"G{h}"))
                wvs.append(wp.tile([P, KD * D], BF, tag=f"Wvp{h}", name=f"Wvp{h}"))
                gcol.append(wp.tile([P, KD], BF, tag=f"gcol{h}", name=f"gcol{h}"))

            def xts(kd, a, b):
                return xt[:, kd * S + a : kd * S + b]

            def gsl(h, kd, a, b):
                return gts[h][:, kd * D + a : kd * D + b]

            def wsl(h, kd):
                return wvs[h][:, kd * D : (kd + 1) * D]

            xT_dv = xT_d[:].rearrange("(k p) s -> p k s", p=P)
            xt_v = xt[:].rearrange("p (k s) -> p k s", k=KD)
            g_v = [g_d[h, :, :].rearrange("(k p) d -> p k d", p=P) for h in range(NHL)]
            gt_v = [gts[h][:].rearrange("p (k d) -> p k d", k=KD) for h in range(NHL)]
            wvp_v = [wvp_d[h, :, :].rearrange("(k p) d -> p k d", p=P) for h in range(NHL)]
            wv_v = [wvs[h][:].rearrange("p (k d) -> p k d", k=KD) for h in range(NHL)]

            # first-needed strips first: G0 columns for m=0, then a half
            # xT chunk, so the first q~ psum group's inputs land ~1us sooner
            nc.sync.dma_start(gt_v[0][:, :, 0:P], g_v[0][:, :, 0:P])
            nc.gpsimd.dma_start(xt_v[:, :, 0 : CH // 2], xT_dv[:, :, 0 : CH // 2])
            nc.sync.dma_start(gt_v[0][:, :, P:D], g_v[0][:, :, P:D])
            nc.gpsimd.dma_start(
                xt_v[:, :, CH // 2 : CH], xT_dv[:, :, CH // 2 : CH]
            )
            for s in range(1, NCH):
                nc.gpsimd.dma_start(
                    xt_v[:, :, s * CH : (s + 1) * CH],
                    xT_dv[:, :, s * CH : (s + 1) * CH],
                )
            xt8 = wp.tile([P, KD * S], FP8, tag="xt8", name="xt8")
            nc.gpsimd.dma_start(
                xt8[:], xT8_d[:].rearrange("(k p) s -> p k s", p=P)
            )
            xt8_v = xt8[:].rearrange("p (k s) -> p k s", k=KD)
            nc.sync.dma_start(wv_v[0], wvp_v[0])
            nc.sync.dma_start(
                gcol[0][:], gv_d[0, :].rearrange("(k p) -> p k", p=P)
            )
            nc.sync.dma_start(gt_v[1], g_v[1])
            nc.sync.dma_start(wv_v[1], wvp_v[1])
            nc.sync.dma_start(gcol[1][:], gv_d[1, :].rearrange("(k p) -> p k", p=P))
            bfvb = []
            for h in range(NHL):
                brow = wp.tile([1, D], F32, tag=f"bfvrow{h}", name=f"bfvrow{h}")
                nc.sync.dma_start(brow[:], bfv_d[h, :].unsqueeze(0))
                bb = wp.tile([P, D], F32, tag=f"bfvb{h}", name=f"bfvb{h}")
                nc.gpsimd.partition_broadcast(bb[:], brow[:])
                bfvb.append(bb)

            # out_acc: head-0 partial, f32 (16 tiles of [128, D])
            out_acc = []
            for st in range(NT):
                out_acc.append(
                    wp.tile([P, D], F32, tag=f"oacc{st}", name=f"oacc{st}")
                )

            for h in range(NHL):
                # --- qtT = G^T @ xT : [D, S] bf16 ------------------------
                qt8 = wp.tile([P, KD * S], FP8, tag="qt8", name=f"qt8_{h}")
                qt8_v = qt8[:].rearrange("p (k s) -> p k s", k=KD)
                qtT_bf = [
                    wp.tile([P, S], BF, tag=f"qtbf{m}", name=f"qtbf{h}_{m}")
                    for m in range(2)
                ]
                for s in range(NCH):
                    for m in range(KD):
                        acc = pp.tile([P, CH], F32, tag="acc", bufs=4)
                        if h == 0 and s == 0 and m == 0:
                            for half in range(2):
                                a0, a1 = half * CH // 2, (half + 1) * CH // 2
                                for kd in range(KD):
                                    nc.tensor.matmul(
                                        acc[:, a0:a1],
                                        gsl(h, kd, 0, P),
                                        xts(kd, a0, a1),
                                        start=(kd == 0),
                                        stop=(kd == KD - 1),
                                    )
                        else:
                            for kd in range(KD):
                                nc.tensor.matmul(
                                    acc[:],
                                    gsl(h, kd, m * P, (m + 1) * P),
                                    xts(kd, s * CH, (s + 1) * CH),
                                    start=(kd == 0),
                                    stop=(kd == KD - 1),
                                )
                        if m < 2:
                            nc.scalar.copy(
                                qt8[:, m * S + s * CH : m * S + (s + 1) * CH],
                                acc[:],
                            )
                        else:
                            nc.scalar.copy(
                                qtT_bf[m - 2][:, s * CH : (s + 1) * CH], acc[:]
                            )

                # --- V' natural [S, D] + t3 bias column ------------------
                v = []
                for t_i in range(NT):
                    v.append(wp.tile([P, D], BF, tag=f"v{t_i}", name=f"v{h}_{t_i}"))
                t3 = wp.tile([P, NT], F32, tag="t3", name=f"t3_{h}", bufs=2)
                for t_i in range(NT):
                    acc = pp.tile([P, D], F32, tag="acc", bufs=4)
                    accb = pp.tile([P, 1], F32, tag="den", bufs=2)
                    for kd in range(KD):
                        nc.tensor.matmul(
                            acc[:],
                            xts(kd, t_i * P, (t_i + 1) * P),
                            wsl(h, kd),
                            start=(kd == 0),
                            stop=(kd == KD - 1),
                        )
                        nc.tensor.matmul(
                            accb[:],
                            xts(kd, t_i * P, (t_i + 1) * P),
                            gcol[h][:, kd : kd + 1],
                            start=(kd == 0),
                            stop=(kd == KD - 1),
                        )
                    nc.vector.tensor_tensor(v[t_i][:], acc[:], bfvb[h][:], Alu.add)
                    nc.scalar.copy(t3[:, t_i : t_i + 1], accb[:])

                # --- attention over sq chunks ----------------------------
                for c in range(NCH):
                    etiles = []
                    for t_i in range(NT):
                        sacc = pp.tile([P, CH], F32, tag="acc", bufs=4)
                        nc.tensor.matmul(
                            sacc[:],
                            xt8_v[:, 0:2, t_i * P : (t_i + 1) * P],
                            qt8_v[:, 0:2, c * CH : (c + 1) * CH],
                            start=True,
                            stop=False,
                            perf_mode=DR,
                        )
                        for kd in (2, 3):
                            nc.tensor.matmul(
                                sacc[:],
                                xts(kd, t_i * P, (t_i + 1) * P),
                                qtT_bf[kd - 2][:, c * CH : (c + 1) * CH],
                                start=False,
                                stop=(kd == 3),
                            )
                        et = wp.tile(
                            [P, CH], BF, tag="E", bufs=3 * NT, name=f"E{h}_{c}_{t_i}"
                        )
                        nc.scalar.activation(
                            et[:],
                            sacc[:],
                            Act.Exp,
                            bias=t3[:, t_i : t_i + 1],
                            scale=INV_SQRT_D,
                        )
                        etiles.append(et)

                    # PV natural + denominator, per 128-row sq tile
                    for j in range(CH // P):
                        st = c * (CH // P) + j
                        pv = pp.tile([P, D], F32, tag="pv", bufs=2)
                        den = pp.tile([P, 1], F32, tag="den", bufs=2)
                        for t_i in range(NT):
                            nc.tensor.matmul(
                                den[:],
                                etiles[t_i][:, j * P : (j + 1) * P],
                                ones_sb[:],
                                start=(t_i == 0),
                                stop=(t_i == NT - 1),
                            )
                            nc.tensor.matmul(
                                pv[:],
                                etiles[t_i][:, j * P : (j + 1) * P],
                                v[t_i][:],
                                start=(t_i == 0),
                                stop=(t_i == NT - 1),
                            )
                        invd = wp.tile([P, 1], F32, tag="invd", bufs=4)
                        nc.vector.reciprocal(invd[:], den[:])
                        if h == 0:
                            nc.vector.tensor_scalar_mul(out_acc[st][:], pv[:], invd[:])
                        else:
                            osb = wp.tile([P, D], F32, tag="osb", bufs=3)
                            nc.vector.scalar_tensor_tensor(
                                osb[:],
                                pv[:],
                                invd[:],
                                out_acc[st][:],
                                Alu.mult,
                                Alu.add,
                            )
                            nc.sync.dma_start(
                                out_d[st * P : (st + 1) * P, :], osb[:]
                            )

    nc.compile()
    return nc


def _get_nc():
    if "nc" not in _CACHE:
        _CACHE["nc"] = _build_nc()
    return _CACHE["nc"]


def _prep_inputs(x, W_sep, b_sep, W_multi, b_multi, W_res, b_res):
    """Host-side exact weight fusion (f64) + sharding + bf16 cast."""
    import ml_dtypes

    bf16 = ml_dtypes.bfloat16
    x = np.asarray(x, dtype=np.float32)
    W_sep = np.asarray(W_sep, dtype=np.float64)
    b_sep = np.asarray(b_sep, dtype=np.float64)
    W_multi = np.asarray(W_multi, dtype=np.float64)
    b_multi = np.asarray(b_multi, dtype=np.float64)
    W_res = np.asarray(W_res, dtype=np.float64)

    W_full = W_multi @ W_sep            # [3*D*H, D]
    b_full = W_multi @ b_sep + b_multi  # [3*D*H]
    Wq = W_full.reshape(H, E3, D)[:, 0:D, :]        # [H, D, D]
    Wk = W_full.reshape(H, E3, D)[:, D : 2 * D, :]
    Wv = W_full.reshape(H, E3, D)[:, 2 * D :, :]
    bq = b_full.reshape(H, E3)[:, 0:D]
    bv = b_full.reshape(H, E3)[:, 2 * D :]
    Wres_h = W_res.reshape(D, H, D).transpose(1, 0, 2)  # [H, dd, d]

    G = np.einsum("hdi,hdj->hij", Wq, Wk)               # [H, D(in), D(in)]
    WvpT = np.einsum("hvi,hdv->hid", Wv, Wres_h)        # [H, D(in), D(dd)]
    gvec = np.einsum("hdi,hd->hi", Wk, bq) * INV_SQRT_D  # [H, D(in)]
    bfv = np.einsum("hdv,hv->hd", Wres_h, bv)            # [H, D(dd)]

    import concourse.mybir as mybir

    fp8 = mybir.dt.np(mybir.dt.float8e4)
    xT = np.ascontiguousarray(x.transpose(0, 2, 1)).astype(bf16)  # [B, D, S]
    xT8 = np.ascontiguousarray(x.transpose(0, 2, 1)).astype(fp8)
    G = np.ascontiguousarray(G).astype(bf16)
    WvpT = np.ascontiguousarray(WvpT).astype(bf16)
    gvec = np.ascontiguousarray(gvec).astype(bf16)
    bfv = np.ascontiguousarray(bfv).astype(np.float32)

    in_maps = []
    for core in range(NC):
        b, hg = divmod(core, 4)
        sl = slice(2 * hg, 2 * hg + 2)
        in_maps.append(
            {
                "xT": xT[b],
                "xT8": xT8[b],
                "G": np.ascontiguousarray(G[sl]),
                "WvpT": np.ascontiguousarray(WvpT[sl]),
                "gvec": np.ascontiguousarray(gvec[sl]),
                "bfv": np.ascontiguousarray(bfv[sl]),
            }
        )
    return in_maps


def kernel(x, W_sep, b_sep, W_multi, b_multi, W_res, b_res):
    from concourse.bass_utils import run_bass_kernel_spmd

    in_maps = _prep_inputs(x, W_sep, b_sep, W_multi, b_multi, W_res, b_res)
    nc = _get_nc()
    res = run_bass_kernel_spmd(nc, in_maps, list(range(NC)), trace=False)

    out = np.zeros((B, S, D), dtype=np.float32)
    for core in range(NC):
        out[core // 4] += res.results[core]["out"]
    out += np.asarray(b_res, dtype=np.float32)
    return out



# revision 5
# speedup vs baseline: 1.6366x; 1.6366x over previous
"""Trainium2 Bass kernel for nn_Attention (B=2, S=2048, D=512, H=8).

Sharding: 8 cores = 2 batches x 4 head-groups (2 heads each).

Algebraic fusions (exact, host-side weight preprocessing in f64):
  W_full = W_multi @ W_sep, G_h = Wq_h^T Wk_h (k-projection vanishes),
  Wvp_h = (Wres_h @ Wv_h)^T (restore matmul vanishes).

Precision scheme: every big matmul runs fp8e4 + DoubleRow (0.5 cy/row,
4x bf16 throughput), with host-side residual splits to recover accuracy:
  x = x8 + xr, G = G8 + Gr, Wvp = W8 + Wr   (all fp8 pairs, exact-ish)
  qtT = G^T x^T      : 3 passes (G8x8 + Grx8 + G8xr)
  V'  = x Wvp        : 2 passes (x8 W8 + xr W8)
  S^T = x qt^T       : 1 pass  (x8 q8), q8 = fp8(qtT psum)
Softmax via shifted weights: E = exp(s/sqrt(D)) = 1 + e. The "1" parts are
exact host-side corrections; only e (small) is quantized to fp8:
  num = c_h + sum_k e8_k V8_k,  c_h = sum_k F_k V'_k (host f64, bf16 pair)
  den = den_c + sum_k e8_k F8_k, den_c = sum_k F_k   (host f64, bf16 pair)
where F = exp(k-side bias) == 1 for zero biases. Both corrections enter as
rank-2 bf16 matmuls that initialize the PSUM accumulation group.
V-side bias and b_res add on host (sum_k w_k = 1 makes this exact).
"""

import numpy as np

P = 128
B = 2
S = 2048
D = 512          # word dim == head dim
H = 8            # total heads
E3 = 3 * D
NHL = 2          # local heads per core
NC = 8           # cores
CH = 512         # sq chunk width
NCH = S // CH    # 4
NT = S // P      # 16 sk tiles
KD = D // P      # 4 contraction k-tiles
INV_SQRT_D = 1.0 / float(np.sqrt(np.float32(D)))
# power-of-2 prescales keeping every fp8 tensor in e4m3's normal range
SX = 8.0         # x
SG = 16.0        # G        (scores carry SG*SX^2 = 1024)
SW = 16.0        # Wvp      (V8/F8/c/den_c carry SX*SW = 128)
S_SCORE = SG * SX * SX

_CACHE = {}


def _build_nc(zero_bias: bool):
    import concourse.mybir as mybir
    import concourse.tile as tile
    from concourse import bacc

    dt = mybir.dt
    BF = dt.bfloat16
    F32 = dt.float32
    FP8 = dt.float8e4
    Act = mybir.ActivationFunctionType
    Alu = mybir.AluOpType
    DR = mybir.MatmulPerfMode.DoubleRow

    nc = bacc.Bacc("TRN2", target_bir_lowering=False, debug=False, num_devices=NC)

    x8_d = nc.declare_dram_parameter("x8", [D, S], FP8, isOutput=False)
    xr_d = nc.declare_dram_parameter("xr", [D, S], FP8, isOutput=False)
    g8_d = nc.declare_dram_parameter("G8", [NHL, D, D], FP8, isOutput=False)
    gr_d = nc.declare_dram_parameter("Gr", [NHL, D, D], FP8, isOutput=False)
    w8_d = nc.declare_dram_parameter("W8", [NHL, D, D], FP8, isOutput=False)
    wr_d = nc.declare_dram_parameter("Wr", [NHL, D, D], FP8, isOutput=False)
    f8_d = nc.declare_dram_parameter("F8", [NHL, S], FP8, isOutput=False)
    cp_d = nc.declare_dram_parameter("cp", [NHL, 2, D], BF, isOutput=False)
    dc_d = nc.declare_dram_parameter("dc", [NHL, 2, 1], BF, isOutput=False)
    if not zero_bias:
        fs_d = nc.declare_dram_parameter("Fs", [NHL, S], F32, isOutput=False)
    out_d = nc.declare_dram_parameter("out", [S, D], F32, isOutput=True)

    with tile.TileContext(nc) as tc:
        with (
            tc.tile_pool(name="w", bufs=1) as wp,
            tc.tile_pool(name="psum", bufs=1, space="PSUM") as pp,
        ):
            ones2 = wp.tile([2, P], BF, tag="ones2")
            nc.vector.memset(ones2[:], 1.0)
            bias0 = wp.tile([P, 1], F32, tag="bias0")
            nc.vector.memset(bias0[:], 0.0)

            x8 = wp.tile([P, KD * S], FP8, tag="x8", name="x8")
            xr = wp.tile([P, KD * S], FP8, tag="xr", name="xr")
            g8s, grs, w8s, wrs, f8s, cps, dcs, fss = [], [], [], [], [], [], [], []
            for h in range(NHL):
                g8s.append(wp.tile([P, KD * D], FP8, tag=f"G8{h}", name=f"G8{h}"))
                grs.append(wp.tile([P, KD * D], FP8, tag=f"Gr{h}", name=f"Gr{h}"))
                w8s.append(wp.tile([P, KD * D], FP8, tag=f"W8{h}", name=f"W8{h}"))
                wrs.append(wp.tile([P, KD * D], FP8, tag=f"Wr{h}", name=f"Wr{h}"))
                f8s.append(wp.tile([P, NT], FP8, tag=f"F8{h}", name=f"F8{h}"))
                cps.append(wp.tile([2, D], BF, tag=f"cp{h}", name=f"cp{h}"))
                dcs.append(wp.tile([2, 1], BF, tag=f"dc{h}", name=f"dc{h}"))
                if not zero_bias:
                    fss.append(wp.tile([P, NT], F32, tag=f"Fs{h}", name=f"Fs{h}"))

            x8_v = x8[:].rearrange("p (k s) -> p k s", k=KD)
            xr_v = xr[:].rearrange("p (k s) -> p k s", k=KD)
            g8_v = [g8s[h][:].rearrange("p (k d) -> p k d", k=KD) for h in range(NHL)]
            gr_v = [grs[h][:].rearrange("p (k d) -> p k d", k=KD) for h in range(NHL)]
            w8_v = [w8s[h][:].rearrange("p (k d) -> p k d", k=KD) for h in range(NHL)]
            wr_v = [wrs[h][:].rearrange("p (k d) -> p k d", k=KD) for h in range(NHL)]
            f8_v = [f8s[h][:].rearrange("p (t u) -> p t u", u=1) for h in range(NHL)]

            x8d_v = x8_d[:].rearrange("(k p) s -> p k s", p=P)
            xrd_v = xr_d[:].rearrange("(k p) s -> p k s", p=P)

            # DMA order: first-needed first, split across sync/gpsimd queues.
            nc.sync.dma_start(g8_v[0], g8_d[0, :, :].rearrange("(k p) d -> p k d", p=P))
            nc.gpsimd.dma_start(x8_v[:, :, 0 : CH // 2], x8d_v[:, :, 0 : CH // 2])
            nc.sync.dma_start(gr_v[0], gr_d[0, :, :].rearrange("(k p) d -> p k d", p=P))
            nc.gpsimd.dma_start(x8_v[:, :, CH // 2 : CH], x8d_v[:, :, CH // 2 : CH])
            for s in range(1, NCH):
                nc.gpsimd.dma_start(
                    x8_v[:, :, s * CH : (s + 1) * CH], x8d_v[:, :, s * CH : (s + 1) * CH]
                )
            for s in range(NCH):
                nc.gpsimd.dma_start(
                    xr_v[:, :, s * CH : (s + 1) * CH], xrd_v[:, :, s * CH : (s + 1) * CH]
                )
            nc.sync.dma_start(w8_v[0], w8_d[0, :, :].rearrange("(k p) d -> p k d", p=P))
            nc.sync.dma_start(wr_v[0], wr_d[0, :, :].rearrange("(k p) d -> p k d", p=P))
            nc.sync.dma_start(g8_v[1], g8_d[1, :, :].rearrange("(k p) d -> p k d", p=P))
            nc.sync.dma_start(gr_v[1], gr_d[1, :, :].rearrange("(k p) d -> p k d", p=P))
            nc.sync.dma_start(w8_v[1], w8_d[1, :, :].rearrange("(k p) d -> p k d", p=P))
            nc.sync.dma_start(wr_v[1], wr_d[1, :, :].rearrange("(k p) d -> p k d", p=P))
            for h in range(NHL):
                nc.sync.dma_start(f8s[h][:], f8_d[h, :].rearrange("(t p) -> p t", p=P))
                nc.sync.dma_start(cps[h][:], cp_d[h, :, :])
                nc.sync.dma_start(dcs[h][:], dc_d[h, :, :])
                if not zero_bias:
                    nc.sync.dma_start(
                        fss[h][:], fs_d[h, :].rearrange("(t p) -> p t", p=P)
                    )

            out_acc = [
                wp.tile([P, D], F32, tag=f"oacc{st}", name=f"oacc{st}")
                for st in range(NT)
            ]

            for h in range(NHL):
                # ---- qtT = G^T x^T, 3 fp8-DR passes, f32 psum -> q8 fp8 ----
                q8 = wp.tile([P, KD * S], FP8, tag="q8", bufs=2, name=f"q8_{h}")
                q8_v = q8[:].rearrange("p (k s) -> p k s", k=KD)
                for s in range(NCH):
                    for mp in range(2):
                        acc = pp.tile([P, 2 * CH], F32, tag="wide", bufs=2)
                        for half in range(2):
                            m = 2 * mp + half
                            ot = acc[:, half * CH : (half + 1) * CH]
                            first = True
                            for ga, xa in ((g8_v, x8_v), (gr_v, x8_v), (g8_v, xr_v)):
                                for kp in range(2):
                                    nc.tensor.matmul(
                                        ot,
                                        ga[h][:, 2 * kp : 2 * kp + 2, m * P : (m + 1) * P],
                                        xa[:, 2 * kp : 2 * kp + 2, s * CH : (s + 1) * CH],
                                        start=first,
                                        stop=(ga is g8_v and xa is xr_v and kp == 1),
                                        perf_mode=DR,
                                    )
                                    first = False
                        nc.scalar.activation(
                            q8_v[:, 2 * mp : 2 * mp + 2, s * CH : (s + 1) * CH],
                            acc[:].rearrange("p (u c) -> p u c", u=2),
                            Act.Copy,
                        )

                # ---- V' = x Wvp, 2 fp8-DR passes -> V8 fp8 -----------------
                v8 = wp.tile([P, NT * D], FP8, tag="v8", bufs=2, name=f"v8_{h}")
                v8_v = v8[:].rearrange("p (t d) -> p t d", t=NT)
                for tp in range(NT // 2):
                    acc = pp.tile([P, 2 * CH], F32, tag="wide", bufs=2)
                    for half in range(2):
                        t = 2 * tp + half
                        ot = acc[:, half * CH : (half + 1) * CH]
                        first = True
                        for xa, wa in ((x8_v, w8_v), (xr_v, w8_v)):
                            for kp in range(2):
                                nc.tensor.matmul(
                                    ot,
                                    xa[:, 2 * kp : 2 * kp + 2, t * P : (t + 1) * P],
                                    wa[h][:, 2 * kp : 2 * kp + 2, :],
                                    start=first,
                                    stop=(xa is xr_v and kp == 1),
                                    perf_mode=DR,
                                )
                                first = False
                    if zero_bias:
                        nc.vector.tensor_copy(
                            v8_v[:, 2 * tp : 2 * tp + 2, :],
                            acc[:].rearrange("p (u c) -> p u c", u=2),
                        )
                    else:
                        for half in range(2):
                            t = 2 * tp + half
                            nc.scalar.activation(
                                v8_v[:, t, :],
                                acc[:, half * CH : (half + 1) * CH],
                                Act.Copy,
                                scale=fss[h][:, t : t + 1],
                            )

                # ---- attention over sq chunks ------------------------------
                for c in range(NCH):
                    e8c = wp.tile([P, NT * CH], FP8, tag="e8c", bufs=2, name=f"e8_{h}_{c}")
                    e8_v = e8c[:].rearrange("p (t s) -> p t s", t=NT)
                    for tp in range(NT // 2):
                        sacc = pp.tile([P, 2 * CH], F32, tag="wide", bufs=2)
                        for half in range(2):
                            t = 2 * tp + half
                            ot = sacc[:, half * CH : (half + 1) * CH]
                            for kp in range(2):
                                nc.tensor.matmul(
                                    ot,
                                    x8_v[:, 2 * kp : 2 * kp + 2, t * P : (t + 1) * P],
                                    q8_v[:, 2 * kp : 2 * kp + 2, c * CH : (c + 1) * CH],
                                    start=(kp == 0),
                                    stop=(kp == 1),
                                    perf_mode=DR,
                                )
                        ebf = wp.tile([P, 2 * CH], F32, tag="ebf", bufs=4)
                        nc.scalar.activation(
                            ebf[:], sacc[:], Act.Exp, bias=bias0[:, 0:1],
                            scale=INV_SQRT_D / S_SCORE,
                        )
                        nc.vector.tensor_scalar_add(
                            e8_v[:, 2 * tp : 2 * tp + 2, :],
                            ebf[:].rearrange("p (u c) -> p u c", u=2),
                            -1.0,
                        )

                    for j in range(CH // P):
                        st = c * (CH // P) + j
                        pv = pp.tile([P, D], F32, tag="pv", bufs=2)
                        den = pp.tile([P, 1], F32, tag="den", bufs=2)
                        nc.tensor.matmul(pv[:], ones2[:], cps[h][:], start=True, stop=False)
                        nc.tensor.matmul(den[:], ones2[:], dcs[h][:], start=True, stop=False)
                        for tp in range(NT // 2):
                            elhs = e8_v[:, 2 * tp : 2 * tp + 2, j * P : (j + 1) * P]
                            nc.tensor.matmul(
                                pv[:], elhs, v8_v[:, 2 * tp : 2 * tp + 2, :],
                                start=False, stop=(tp == NT // 2 - 1), perf_mode=DR,
                            )
                            nc.tensor.matmul(
                                den[:], elhs, f8_v[h][:, 2 * tp : 2 * tp + 2, :],
                                start=False, stop=(tp == NT // 2 - 1), perf_mode=DR,
                            )
                        invd = wp.tile([P, 1], F32, tag="invd", bufs=4)
                        nc.vector.reciprocal(invd[:], den[:])
                        if h == 0:
                            nc.vector.tensor_scalar_mul(out_acc[st][:], pv[:], invd[:])
                        else:
                            osb = wp.tile([P, D], F32, tag="osb", bufs=3)
                            nc.vector.scalar_tensor_tensor(
                                osb[:], pv[:], invd[:], out_acc[st][:],
                                Alu.mult, Alu.add,
                            )
                            nc.sync.dma_start(out_d[st * P : (st + 1) * P, :], osb[:])

    nc.compile()
    return nc


def _get_nc(zero_bias: bool = True):
    key = ("nc", zero_bias)
    if key not in _CACHE:
        _CACHE[key] = _build_nc(zero_bias)
    return _CACHE[key]


def _prep_inputs(x, W_sep, b_sep, W_multi, b_multi, W_res, b_res):
    """Host-side exact weight fusion (f64) + fp8 residual splits + sharding."""
    import ml_dtypes
    import concourse.mybir as mybir

    bf16 = ml_dtypes.bfloat16
    fp8 = mybir.dt.np(mybir.dt.float8e4)

    x = np.asarray(x, dtype=np.float64)
    W_sep = np.asarray(W_sep, dtype=np.float64)
    b_sep = np.asarray(b_sep, dtype=np.float64)
    W_multi = np.asarray(W_multi, dtype=np.float64)
    b_multi = np.asarray(b_multi, dtype=np.float64)
    W_res = np.asarray(W_res, dtype=np.float64)

    zero_bias = not (np.any(b_sep) or np.any(b_multi))

    W_full = W_multi @ W_sep            # [3*D*H, D]
    b_full = W_multi @ b_sep + b_multi  # [3*D*H]
    Wq = W_full.reshape(H, E3, D)[:, 0:D, :]
    Wk = W_full.reshape(H, E3, D)[:, D : 2 * D, :]
    Wv = W_full.reshape(H, E3, D)[:, 2 * D :, :]
    bq = b_full.reshape(H, E3)[:, 0:D]
    bv = b_full.reshape(H, E3)[:, 2 * D :]
    Wres_h = W_res.reshape(D, H, D).transpose(1, 0, 2)   # [H, dd, d]

    G = np.einsum("hdi,hdj->hij", Wq, Wk)                # [H, Din, Din]
    WvpT = np.einsum("hvi,hdv->hid", Wv, Wres_h)         # [H, Din, Ddd]
    gvec = np.einsum("hdi,hd->hi", Wk, bq) * INV_SQRT_D  # [H, Din]
    bfv = np.einsum("hdv,hv->hd", Wres_h, bv)            # [H, Ddd]

    def split8(a):
        a8 = a.astype(fp8)
        ar = (a - a8.astype(np.float64)).astype(fp8)
        return np.ascontiguousarray(a8), np.ascontiguousarray(ar)

    xT = x.transpose(0, 2, 1)                            # [B, D, S]
    x8, xr = split8(xT * SX)
    G8, Gr = split8(G * SG)
    W8, Wr = split8(WvpT * SW)

    t3 = np.einsum("bsd,hd->bhs", x, gvec)               # [B, H, S]
    F = np.exp(t3)
    F8 = np.ascontiguousarray((F * (SX * SW)).astype(fp8))
    den_c = F.sum(axis=2) * (SX * SW)                    # [B, H]
    # c[b,h,d] = sum_s F[b,h,s] * (x[b] @ WvpT[h])[s,d]  (no v-bias)
    Fx = np.einsum("bhs,bsd->bhd", F, x)                 # [B, H, Din]
    c = np.einsum("bhd,hdi->bhi", Fx, WvpT) * (SX * SW)  # [B, H, Ddd]

    def bfpair(a):
        a1 = a.astype(bf16)
        a2 = (a - a1.astype(np.float64)).astype(bf16)
        return np.stack([a1, a2], axis=-2)               # [..., 2, D]

    cp = bfpair(c)                                       # [B, H, 2, D]
    dc = bfpair(den_c[..., None])                        # [B, H, 2, 1]

    host_bias = bfv.sum(axis=0) + np.asarray(b_res, dtype=np.float64)  # [D]

    in_maps = []
    for core in range(NC):
        b, hg = divmod(core, 4)
        sl = slice(2 * hg, 2 * hg + 2)
        m = {
            "x8": x8[b],
            "xr": xr[b],
            "G8": G8[sl],
            "Gr": Gr[sl],
            "W8": W8[sl],
            "Wr": Wr[sl],
            "F8": np.ascontiguousarray(F8[b, sl]),
            "cp": np.ascontiguousarray(cp[b, sl]),
            "dc": np.ascontiguousarray(dc[b, sl]),
        }
        if not zero_bias:
            m["Fs"] = np.ascontiguousarray(F[b, sl].astype(np.float32))
        in_maps.append(m)
    return in_maps, host_bias, zero_bias


def kernel(x, W_sep, b_sep, W_multi, b_multi, W_res, b_res):
    from concourse.bass_utils import run_bass_kernel_spmd

    in_maps, host_bias, zero_bias = _prep_inputs(
        x, W_sep, b_sep, W_multi, b_multi, W_res, b_res
    )
    nc = _get_nc(zero_bias)
    res = run_bass_kernel_spmd(nc, in_maps, list(range(NC)), trace=False)

    out = np.zeros((B, S, D), dtype=np.float64)
    for core in range(NC):
        out[core // 4] += np.asarray(res.results[core]["out"], dtype=np.float64)
    out += host_bias
    return out.astype(np.float32)
